# revision 9
# baseline (speedup 1.0000x reference)
"""LoRA linear kernel for Trainium2 (Bass/Tile), 8-core SPMD.

Computes out = x @ (A @ B) * (alpha/r) for
  x: [4, 4096, 4096] f32, A: [4096, 16] f32, B: [16, 4096] f32
with alpha/r == 1.0.

Algorithm: reassociate as out = (x @ A) @ B -- 256x fewer FLOPs than
materializing the 4096x4096 delta-weight.  Data-parallel over rows of x:
each of the 8 cores gets 2048 rows.

Device work is two matmul phases with NO on-device transposes:
the host pre-transposes each core's x shard to xT [K, M_shard] in fp16
(rel-err budget is 2e-2; fp16 rounding contributes ~5e-4), so the
contraction dim k is already on partitions.

  phase 1: tT[r, m] += A_chunk[k,r].T @ xT[k, m]   (accum over 32 k-chunks)
  phase 2: out[m, n] = tT[:, m].T @ B[r, n]        (per 128-row m-tile)

fp16 halves HBM traffic vs the f32 baseline: 16.8 MB in + 16.8 MB out
per core ~= 94 us at the 358 GB/s per-core HBM limit.

A dummy-matmul warmup burst at kernel start keeps the PE HAM clock gate
at 8/8 (2.4 GHz): without it every matmul runs at the cold 1.2 GHz rate
(427 ns instead of 215 ns for N=512), which makes both phases PE-bound
instead of DMA-bound.
"""

import os
import sys

import numpy as np

for _p in ("/opt/trn_rl_repo",):
    if os.path.isdir(_p) and _p not in sys.path:
        sys.path.insert(0, _p)

import concourse.bacc as bacc
import concourse.bass as bass
import concourse.mybir as mybir
from concourse import tile
from concourse.bass_utils import run_bass_kernel_spmd

R = 16
B_DIM = 4
SEQ = 4096
K = 4096  # in_features
N = 4096  # out_features
M_FULL = B_DIM * SEQ  # 16384
NCORES = 8
M_SHARD = M_FULL // NCORES  # 2048
SCALING = 16.0 / 16.0  # alpha / r == 1.0

KC = 128  # contraction chunk (partition dim)
N_KC = K // KC  # 32
NB = 512  # one PSUM bank of fp32
MQ = M_SHARD // NB  # 4 phase-1 bank quarters
MT = 128  # rows per phase-2 m-tile
N_MT = M_SHARD // MT  # 16
N_NB = N // NB  # 8
N_WARM = 12  # dummy matmuls to lift the HAM clock gate (needs ~3.4us busy)

_F32 = mybir.dt.float32
_F16 = mybir.dt.float16


def _build_kernel(tc, nc, xt, a_pre, b_in, out):
    with (
        tc.tile_pool(name="const", bufs=1) as cpool,
        tc.tile_pool(name="xin", bufs=6) as xpool,
        tc.tile_pool(name="tps", bufs=1, space="PSUM") as tpsum,
        tc.tile_pool(name="ops", bufs=4, space="PSUM") as opsum,
        tc.tile_pool(name="osb", bufs=3) as opool,
    ):
        # A pre-arranged on host to [128, n_kc * R]: col block c holds
        # A[c*128:(c+1)*128, :] with k on partitions.
        a_sb = cpool.tile([128, N_KC * R], _F16, name="a_sb")
        nc.sync.dma_start(out=a_sb, in_=a_pre)
        # B and t are zero-padded from 16 to 128 contraction rows: matmul
        # cost depends only on the streamed column count, but a K=128
        # matmul registers as PE activity for the HAM clock gate while a
        # K=16 one does not (16/128 rows active reads as idle -> the PE
        # would run phase 2 at the cold 1.2 GHz rate).
        b_sb = cpool.tile([128, N], _F16, name="b_sb")
        nc.gpsimd.memset(b_sb[:], 0.0)
        nc.sync.dma_start(out=b_sb[0:R, :], in_=b_in)

        tps = [tpsum.tile([R, NB], _F32, name=f"t{q}") for q in range(MQ)]

        # PE warmup: back-to-back dummy matmuls while the first x DMAs are
        # in flight.  They overwrite tps[0], which phase 1 re-clears via
        # start=True on its first accumulating matmul.
        warm = cpool.tile([128, NB], _F16, name="warm")
        nc.gpsimd.memset(warm[:], 0.0)
        for _ in range(N_WARM):
            nc.tensor.matmul(
                tps[0][:], warm[:, 0:R], warm[:], start=True, stop=True
            )

        # Phase 1: tT[r, m] = sum_c A_c.T @ xT_c, m split in 4 bank quarters.
        for c in range(N_KC):
            xtile = xpool.tile([KC, M_SHARD], _F16)
            nc.sync.dma_start(out=xtile, in_=xt[c * KC : (c + 1) * KC, :])
            for q in range(MQ):
                nc.tensor.matmul(
                    tps[q][:],
                    a_sb[:, c * R : (c + 1) * R],
                    xtile[:, q * NB : (q + 1) * NB],
                    start=(c == 0),
                    stop=(c == N_KC - 1),
                )

        t_sb = cpool.tile([128, M_SHARD], _F16, name="t_sb")
        nc.gpsimd.memset(t_sb[:], 0.0)
        for q in range(MQ):
            dst = t_sb[0:R, q * NB : (q + 1) * NB]
            if q % 2 == 0:
                nc.vector.tensor_copy(dst, tps[q][:])
            else:
                nc.scalar.copy(dst, tps[q][:])

        # Phase 2: out[m, n] = tT[:, m].T @ B, per 128-row m-tile.
        for m in range(N_MT):
            osb = opool.tile([MT, N], _F16)
            for j in range(N_NB):
                ops = opsum.tile([MT, NB], _F32)
                nc.tensor.matmul(
                    ops[:],
                    t_sb[:, m * MT : (m + 1) * MT],
                    b_sb[:, j * NB : (j + 1) * NB],
                    start=True,
                    stop=True,
                )
                dst = osb[:, j * NB : (j + 1) * NB]
                if j % 2 == 0:
                    nc.vector.tensor_copy(dst, ops[:])
                else:
                    nc.scalar.copy(dst, ops[:])
            nc.sync.dma_start(out=out[m * MT : (m + 1) * MT, :], in_=osb[:])


_NC_CACHE = None


def _get_nc():
    global _NC_CACHE
    if _NC_CACHE is not None:
        return _NC_CACHE
    nc = bacc.Bacc("TRN2", target_bir_lowering=False, debug=False)
    xt = nc.dram_tensor("xt", [K, M_SHARD], _F16, kind="ExternalInput").ap()
    a_pre = nc.dram_tensor("a_pre", [128, N_KC * R], _F16, kind="ExternalInput").ap()
    b_in = nc.dram_tensor("b_in", [R, N], _F16, kind="ExternalInput").ap()
    out = nc.dram_tensor("out", [M_SHARD, N], _F16, kind="ExternalOutput").ap()
    with tile.TileContext(nc) as tc:
        _build_kernel(tc, nc, xt, a_pre, b_in, out)
    nc.compile()
    _NC_CACHE = nc
    return nc


LAST_RESULTS = None


def kernel(x: np.ndarray, A: np.ndarray, B: np.ndarray) -> np.ndarray:
    global LAST_RESULTS
    assert x.shape == (B_DIM, SEQ, K), x.shape
    assert A.shape == (K, R), A.shape
    assert B.shape == (R, N), B.shape

    x16 = np.asarray(x, dtype=np.float32).reshape(M_FULL, K).astype(np.float16)
    a_np = np.asarray(A, dtype=np.float32).astype(np.float16)
    b_np = (np.asarray(B, dtype=np.float32) * SCALING).astype(np.float16)

    # Host pre-arrangement of A: [K, R] -> [128, (K/128) * R]
    a_pre = np.ascontiguousarray(
        a_np.reshape(K // KC, KC, R).transpose(1, 0, 2).reshape(128, N_KC * R)
    )

    in_maps = []
    for i in range(NCORES):
        in_maps.append(
            {
                "xt": np.ascontiguousarray(x16[i * M_SHARD : (i + 1) * M_SHARD].T),
                "a_pre": a_pre,
                "b_in": b_np,
            }
        )

    nc = _get_nc()
    trace = os.environ.get("KERNEL_TRACE", "0") == "1"
    tmpdir = os.environ.get("KERNEL_TMPDIR") or None
    res = run_bass_kernel_spmd(
        nc, in_maps, core_ids=list(range(NCORES)), trace=trace, tmpdir=tmpdir
    )
    LAST_RESULTS = res
    out = np.concatenate([res.results[i]["out"] for i in range(NCORES)], axis=0)
    return out.reshape(B_DIM, SEQ, N).astype(np.float32)
